# revision 7
# baseline (speedup 1.0000x reference)
"""Binarize kernel for Trainium2: out[b, d, n/8] = packbits(x[b, :] > th[d]).

x: [2048, 32768] f32. depth_ths: [3] f32. out: [2048, 3, 4096] uint8.
8-way data parallel over batch (256 rows/core).

Architecture (v2 — engine-balanced; measured 1.25-1.5x faster than the
v1 all-stride-8 matmul bitpack, quiet-chip loop-slope ~98 us/core vs
~138, contended medians ~131-157 vs ~197):
  The kernel is NOT DMA-bound (DMA floor ~65 us/core vs ~200 us for the
  v1 matmul-bitpack): the binding resources are the elementwise engines
  (DVE ~274 G elem/s contiguous f32 compares but ~149 G strided; ACT
  ~145 G) and the PE whose stride-8 fp8 moving-operand fetch runs at
  ~1 col/cycle vs ~2 cols/cycle contiguous.

  Measured-rate-balanced assignment per [128, 8192] x-tile:
  - planes 0/2 (th=-0.67/+0.67): "deinterleaved sub-compares" — 8
    tensor_scalar is_gt ops per plane with strided f32 reads (full port
    rate) writing contiguous fp8 segments bits[p, i*1024:(i+1)*1024] =
    bit i of each output byte. A few segments (i >= ndve) go to ACT as
    Sign ({-1,+1}, halved matmul weight 2^(6-i), constant folded into
    the PSUM drain) to offload DVE. Matmul moving operands are then
    CONTIGUOUS -> PE ~2.3x faster than stride-8.
  - plane 1 (th=0.0): contiguous ACT Sign over the whole tile (ACT's
    cheapest form), stride-8 matmuls with halved weights + 0.5S+127.5
    drain. PE budget has room for one stride-8 plane.
  - PSUM drains split across ACT/DVE per plane.
  Weights (scaled identities, fp8) are generated on-chip via gpsimd
  memset + affine_select; no weight input tensor.
"""

import sys

import numpy as np

try:
    from concourse import bacc, bass, mybir, tile
    from concourse.bass_utils import run_bass_kernel_spmd
except ImportError:  # fresh grading dir: concourse lives in the trn repo
    sys.path.insert(0, "/opt/trn_rl_repo")
    from concourse import bacc, bass, mybir, tile
    from concourse.bass_utils import run_bass_kernel_spmd

B, N = 2048, 32768
NCORES = 8
ROWS = B // NCORES          # 256 rows per core
NB = N // 8                 # 4096 output bytes per row per threshold
P = 128                     # partitions
FT = 8192                   # free-dim tile of x (f32) per inner iteration
GT = FT // 8                # bytes per row per tile-plane = 1024
CHUNK = 512                 # matmul free dim (one PSUM bank)

# per-plane scheme: int n -> deinterleaved, i < n DVE is_gt / i >= n ACT
# Sign; "act_contig" -> contiguous ACT Sign plane (stride-8 matmuls).
CMP = (7, "act_contig", 7)
DRAIN = ("act", "act", "act")   # PSUM drain engine per plane (DVE is the
                                # binding engine; keep all drains off it)
XBUFS, BBUFS, OBUFS, PSBUFS = 2, 6, 2, 4  # psum tiles are 2 banks each

_cache: dict = {}


def _build(
    ths,
    loop: int = 1,
    cmp_eng=CMP,
    drain_eng=DRAIN,
    xbufs: int = XBUFS,
    bbufs: int = BBUFS,
    obufs: int = OBUFS,
    psbufs: int = PSBUFS,
) -> "bass.Bass":
    nc = bacc.Bacc()
    # const APs for ACT Sign biases (only 0.0/1.0 pre-registered by bacc)
    for th in sorted({-float(t) for t in ths} - {0.0, 1.0}):
        cts = nc.alloc_sbuf_tensor(f"const-f32-{th}", [P, 1], mybir.dt.float32)
        nc.gpsimd.memset(cts.ap(), th)
        nc.const_aps.aps[(mybir.dt.float32, th)] = cts.ap()
    nc.all_engine_barrier()

    x_in = nc.declare_dram_parameter("x", [ROWS, N], mybir.dt.float32, isOutput=False)
    out_ext = nc.declare_dram_parameter(
        "out", [ROWS, 3, NB], mybir.dt.uint8, isOutput=True
    )
    out_flat = out_ext.ap().rearrange("r d g -> r (d g)")

    def make_w(wtile):
        # blocks 0-7: 2^(7-i) * I ({0,1} bits); 8-15: 2^(6-i) * I (Sign)
        for b in range(16):
            s = float(2 ** (7 - b)) if b < 8 else float(2.0 ** (6 - (b - 8)))
            blk = wtile[:, b * P : (b + 1) * P]
            nc.gpsimd.memset(blk, s)
            nc.gpsimd.affine_select(
                out=blk, in_=blk, pattern=[[1, P]],
                compare_op=mybir.AluOpType.is_equal, fill=0.0,
                base=0, channel_multiplier=-1,
            )

    def body(tc, wtile, xpool, bpool, opool, pspool):
        for pb in range(ROWS // P):
            r0 = pb * P
            ob = opool.tile([P, 3 * NB], mybir.dt.uint8, name="ob", tag="ob")
            for fti in range(N // FT):
                c0 = fti * FT
                g0 = c0 // 8
                nchunks = GT // CHUNK
                xt = xpool.tile([P, FT], mybir.dt.float32, name="xt", tag="xt")
                nc.sync.dma_start(out=xt[:], in_=x_in[r0 : r0 + P, c0 : c0 + FT])

                bvs, ndve = [], []
                for t in range(3):
                    bits = bpool.tile(
                        [P, FT], mybir.dt.float8e4, name="bits", tag="bits"
                    )
                    ce = cmp_eng[t]
                    if ce == "act_contig":
                        ndve.append(-1)
                        nc.scalar.activation(
                            out=bits[:], in_=xt[:],
                            func=mybir.ActivationFunctionType.Sign,
                            bias=-ths[t],
                        )
                        bvs.append(
                            bits.rearrange("p (c g e) -> p c g e", g=CHUNK, e=8)
                        )
                        continue
                    n = int(ce)
                    ndve.append(n)
                    xv = xt.rearrange("p (g e) -> p e g", e=8)
                    for i in range(8):
                        dst = bits[:, i * GT : (i + 1) * GT]
                        if i >= n:
                            nc.scalar.activation(
                                out=dst, in_=xv[:, i, :],
                                func=mybir.ActivationFunctionType.Sign,
                                bias=-ths[t],
                            )
                        else:
                            nc.vector.tensor_scalar(
                                out=dst, in0=xv[:, i, :], scalar1=ths[t],
                                scalar2=None, op0=mybir.AluOpType.is_gt,
                            )
                    bvs.append(
                        bits.rearrange("p (e c g) -> p e c g", e=8, g=CHUNK)
                    )

                # one 2-bank PSUM tile per plane; matmuls write bank-aligned
                # 512-col halves; drain is a single merged [P, 1024] op
                pss = {
                    t: pspool.tile([P, GT], mybir.dt.float32, name="ps", tag="ps")
                    for t in range(3)
                }
                for i in range(8):
                    # group by weight block to minimize LDWEIGHTS switches
                    ts_order = sorted(range(3), key=lambda t: 0 <= ndve[t] <= i)
                    for t in ts_order:
                        for c in range(nchunks):
                            if ndve[t] >= 0:
                                mv = bvs[t][:, i, c, :]     # contiguous
                            else:
                                mv = bvs[t][:, c, :, i]     # stride-8
                            wb = i
                            if 0 <= ndve[t] <= i or ndve[t] < 0:
                                wb = 8 + i  # halved weights for Sign bits
                            nc.tensor.matmul(
                                pss[t][:, c * CHUNK : (c + 1) * CHUNK],
                                wtile[:, wb * P : (wb + 1) * P],
                                mv,
                                start=(i == 0),
                                stop=(i == 7),
                            )
                for t, ps in pss.items():
                    o0 = t * NB + g0
                    dst = ob[:, o0 : o0 + GT]
                    eng = drain_eng[t]
                    if ndve[t] < 0:
                        C = 127.5  # all-Sign plane, halved weights
                    else:
                        C = sum(2.0 ** (6 - i) for i in range(ndve[t], 8))
                    if C == 0.0:
                        if eng == "dve":
                            nc.vector.tensor_copy(out=dst, in_=ps[:])
                        else:
                            nc.scalar.copy(out=dst, in_=ps[:])
                    elif eng == "dve":
                        nc.vector.tensor_scalar(
                            out=dst, in0=ps[:], scalar1=C, scalar2=None,
                            op0=mybir.AluOpType.add,
                        )
                    else:
                        nc.scalar.activation(
                            out=dst, in_=ps[:],
                            func=mybir.ActivationFunctionType.Copy,
                            bias=C,
                        )
            nc.sync.dma_start(out=out_flat[r0 : r0 + P, :], in_=ob[:])

    with tile.TileContext(nc) as tc:
        with (
            tc.tile_pool(name="wpool", bufs=1) as wpool,
            tc.tile_pool(name="xpool", bufs=xbufs) as xpool,
            tc.tile_pool(name="bpool", bufs=bbufs) as bpool,
            tc.tile_pool(name="opool", bufs=obufs) as opool,
            tc.tile_pool(name="psum", bufs=psbufs, space="PSUM") as pspool,
        ):
            wtile = wpool.tile([P, 16 * P], mybir.dt.float8e4)
            make_w(wtile)
            if loop == 1:
                body(tc, wtile, xpool, bpool, opool, pspool)
            else:
                with tc.For_i(0, loop, 1):
                    body(tc, wtile, xpool, bpool, opool, pspool)
    nc.compile()
    return nc


def kernel(x: np.ndarray, depth_ths: np.ndarray) -> np.ndarray:
    x = np.asarray(x)
    ths = tuple(float(v) for v in np.asarray(depth_ths, dtype=np.float32))
    assert x.shape == (B, N) and len(ths) == 3

    if ths not in _cache:
        _cache[ths] = _build(ths)
    nc = _cache[ths]

    in_maps = [
        {"x": np.ascontiguousarray(x[i * ROWS : (i + 1) * ROWS])}
        for i in range(NCORES)
    ]
    res = run_bass_kernel_spmd(nc, in_maps, list(range(NCORES)))
    return np.concatenate([res.results[i]["out"] for i in range(NCORES)], axis=0)


# revision 8
# speedup vs baseline: 1.0273x; 1.0273x over previous
"""Binarize kernel for Trainium2: out[b, d, n/8] = packbits(x[b, :] > th[d]).

x: [2048, 32768] f32. depth_ths: [3] f32. out: [2048, 3, 4096] uint8.
8-way data parallel over batch (256 rows/core).

Architecture (v4 — DoubleRow bitpack; measured best of 4 generations):
  The kernel is NOT DMA-bound (DMA floor ~60-90 us/core): the binding
  resources are the elementwise engines and the PE moving-operand fetch.
  - Compares stay CONTIGUOUS (fastest form on both engines): planes
    0/2 (th=-0.67/+0.67) on DVE tensor_scalar is_gt (~274 G elem/s,
    2x_2P mode), plane 1 (th=0.0) on ACT Sign (~145 G; {-1,+1}
    encoding, halved matmul weights + 127.5 folded into the drain).
  - Bit-packing: byte[g] = sum_i 2^(7-i) bits[8g+i] via PE matmuls with
    scaled-identity fp8 stationary weights. perf_mode=DoubleRow packs
    the two ADJACENT bits (2j, 2j+1) of each byte per PE cell: 4
    DoubleRow matmuls per 512-byte chunk instead of 8 stride-8 matmuls
    (which fetch at only ~1 col/cycle). Stationary = [w_2j || w_2j+1]
    viewed [p, 2, 128]; moving = bit-pairs viewed [p, 2, 512].
    Measured -14% end-to-end vs the deinterleaved-sub-compare v3.
  - PSUM: one 2-bank [128, 1024] f32 tile per plane (matmuls write
    bank-aligned halves), drained by a single merged op per plane, all
    drains on ACT (DVE is the binding engine).
  Weights are generated on-chip via gpsimd memset + affine_select; no
  weight input tensor. ACT Sign with nonzero bias needs manually
  registered const APs (only 0.0/1.0 are pre-registered).
  Requires no x == th exactly (holds for this input distribution).
"""

import sys

import numpy as np

try:
    from concourse import bacc, bass, mybir, tile
    from concourse.bass_utils import run_bass_kernel_spmd
except ImportError:  # fresh grading dir: concourse lives in the trn repo
    sys.path.insert(0, "/opt/trn_rl_repo")
    from concourse import bacc, bass, mybir, tile
    from concourse.bass_utils import run_bass_kernel_spmd

B, N = 2048, 32768
NCORES = 8
ROWS = B // NCORES          # 256 rows per core
NB = N // 8                 # 4096 output bytes per row per threshold
P = 128                     # partitions
FT = 8192                   # free-dim tile of x (f32) per inner iteration
GT = FT // 8                # bytes per row per tile-plane = 1024
CHUNK = 512                 # matmul free dim (one PSUM bank)

DRAIN = ("act", "act", "act")   # PSUM drain engine per plane
XBUFS, BBUFS, OBUFS, PSBUFS = 2, 6, 2, 4  # psum tiles are 2 banks each

_cache: dict = {}


def _build(
    ths,
    loop: int = 1,
    drain_eng=DRAIN,
    xbufs: int = XBUFS,
    bbufs: int = BBUFS,
    obufs: int = OBUFS,
    psbufs: int = PSBUFS,
) -> "bass.Bass":
    nc = bacc.Bacc()
    # const APs for ACT Sign biases (only 0.0/1.0 pre-registered by bacc)
    for th in sorted({-float(t) for t in ths} - {0.0, 1.0}):
        cts = nc.alloc_sbuf_tensor(f"const-f32-{th}", [P, 1], mybir.dt.float32)
        nc.gpsimd.memset(cts.ap(), th)
        nc.const_aps.aps[(mybir.dt.float32, th)] = cts.ap()
    nc.all_engine_barrier()

    x_in = nc.declare_dram_parameter("x", [ROWS, N], mybir.dt.float32, isOutput=False)
    out_ext = nc.declare_dram_parameter(
        "out", [ROWS, 3, NB], mybir.dt.uint8, isOutput=True
    )
    out_flat = out_ext.ap().rearrange("r d g -> r (d g)")

    def make_w(wtile):
        # 16 half-blocks of 128 cols; DoubleRow pairs consecutive halves.
        # blocks 0-7: 2^(7-i) * I ({0,1} bits); 8-15: 2^(6-i) * I (Sign).
        for b in range(16):
            s = float(2 ** (7 - b)) if b < 8 else float(2.0 ** (6 - (b - 8)))
            blk = wtile[:, b * P : (b + 1) * P]
            nc.gpsimd.memset(blk, s)
            nc.gpsimd.affine_select(
                out=blk, in_=blk, pattern=[[1, P]],
                compare_op=mybir.AluOpType.is_equal, fill=0.0,
                base=0, channel_multiplier=-1,
            )

    def body(tc, wtile, xpool, bpool, opool, pspool):
        wv = wtile.rearrange("p (j h m) -> p j h m", j=8, h=2)  # DR pair view
        for pb in range(ROWS // P):
            r0 = pb * P
            ob = opool.tile([P, 3 * NB], mybir.dt.uint8, name="ob", tag="ob")
            for fti in range(N // FT):
                c0 = fti * FT
                g0 = c0 // 8
                nchunks = GT // CHUNK
                xt = xpool.tile([P, FT], mybir.dt.float32, name="xt", tag="xt")
                nc.sync.dma_start(out=xt[:], in_=x_in[r0 : r0 + P, c0 : c0 + FT])

                bvs = []
                for t in range(3):
                    bits = bpool.tile(
                        [P, FT], mybir.dt.float8e4, name="bits", tag="bits"
                    )
                    if t == 1:
                        nc.scalar.activation(
                            out=bits[:], in_=xt[:],
                            func=mybir.ActivationFunctionType.Sign,
                            bias=-ths[t],
                        )
                    else:
                        nc.vector.tensor_scalar(
                            out=bits[:], in0=xt[:], scalar1=ths[t],
                            scalar2=None, op0=mybir.AluOpType.is_gt,
                        )
                    # bit-pair view: [p, chunk, pairidx j, pair elem, byte]
                    bvs.append(
                        bits.rearrange("p (c g f e) -> p c f e g", g=CHUNK, f=4, e=2)
                    )

                # one 2-bank PSUM tile per plane; single merged drain
                pss = {
                    t: pspool.tile([P, GT], mybir.dt.float32, name="ps", tag="ps")
                    for t in range(3)
                }
                for j in range(4):
                    for t in range(3):
                        enc = 4 if t == 1 else 0  # Sign weight blocks at 4+j
                        for c in range(nchunks):
                            nc.tensor.matmul(
                                pss[t][:, c * CHUNK : (c + 1) * CHUNK],
                                wv[:, enc + j, :, :],
                                bvs[t][:, c, j, :, :],
                                start=(j == 0), stop=(j == 3),
                                perf_mode=mybir.MatmulPerfMode.DoubleRow,
                            )
                for t, ps in pss.items():
                    dst = ob[:, t * NB + g0 : t * NB + g0 + GT]
                    eng = drain_eng[t]
                    C = 127.5 if t == 1 else 0.0  # Sign-plane offset
                    if C == 0.0:
                        if eng == "dve":
                            nc.vector.tensor_copy(out=dst, in_=ps[:])
                        else:
                            nc.scalar.copy(out=dst, in_=ps[:])
                    elif eng == "dve":
                        nc.vector.tensor_scalar(
                            out=dst, in0=ps[:], scalar1=C, scalar2=None,
                            op0=mybir.AluOpType.add,
                        )
                    else:
                        nc.scalar.activation(
                            out=dst, in_=ps[:],
                            func=mybir.ActivationFunctionType.Copy,
                            bias=C,
                        )
            nc.sync.dma_start(out=out_flat[r0 : r0 + P, :], in_=ob[:])

    with tile.TileContext(nc) as tc:
        with (
            tc.tile_pool(name="wpool", bufs=1) as wpool,
            tc.tile_pool(name="xpool", bufs=xbufs) as xpool,
            tc.tile_pool(name="bpool", bufs=bbufs) as bpool,
            tc.tile_pool(name="opool", bufs=obufs) as opool,
            tc.tile_pool(name="psum", bufs=psbufs, space="PSUM") as pspool,
        ):
            wtile = wpool.tile([P, 16 * P], mybir.dt.float8e4)
            make_w(wtile)
            if loop == 1:
                body(tc, wtile, xpool, bpool, opool, pspool)
            else:
                with tc.For_i(0, loop, 1):
                    body(tc, wtile, xpool, bpool, opool, pspool)
    nc.compile()
    return nc


def kernel(x: np.ndarray, depth_ths: np.ndarray) -> np.ndarray:
    x = np.asarray(x)
    ths = tuple(float(v) for v in np.asarray(depth_ths, dtype=np.float32))
    assert x.shape == (B, N) and len(ths) == 3

    if ths not in _cache:
        _cache[ths] = _build(ths)
    nc = _cache[ths]

    in_maps = [
        {"x": np.ascontiguousarray(x[i * ROWS : (i + 1) * ROWS])}
        for i in range(NCORES)
    ]
    res = run_bass_kernel_spmd(nc, in_maps, list(range(NCORES)))
    return np.concatenate([res.results[i]["out"] for i in range(NCORES)], axis=0)


# revision 9
# speedup vs baseline: 1.1653x; 1.1344x over previous
"""Binarize kernel for Trainium2: out[b, d, n/8] = packbits(x[b, :] > th[d]).

x: [2048, 32768] f32. depth_ths: [3] f32. out: [2048, 3, 4096] uint8.
8-way data parallel over batch (256 rows/core).

Architecture (v4 — DoubleRow bitpack; measured best of 4 generations):
  The kernel is NOT DMA-bound (DMA floor ~60-90 us/core): the binding
  resources are the elementwise engines and the PE moving-operand fetch.
  - Compares stay CONTIGUOUS (fastest form on both engines): planes
    0/2 (th=-0.67/+0.67) on DVE tensor_scalar is_gt (~274 G elem/s,
    2x_2P mode), plane 1 (th=0.0) on ACT Sign (~145 G; {-1,+1}
    encoding, halved matmul weights + 127.5 folded into the drain).
  - Bit-packing: byte[g] = sum_i 2^(7-i) bits[8g+i] via PE matmuls with
    scaled-identity fp8 stationary weights. perf_mode=DoubleRow packs
    the two ADJACENT bits (2j, 2j+1) of each byte per PE cell: 4
    DoubleRow matmuls per 512-byte chunk instead of 8 stride-8 matmuls
    (which fetch at only ~1 col/cycle). Stationary = [w_2j || w_2j+1]
    viewed [p, 2, 128]; moving = bit-pairs viewed [p, 2, 512].
    Measured -14% end-to-end vs the deinterleaved-sub-compare v3.
  - PSUM: one 2-bank [128, 1024] f32 tile per plane (matmuls write
    bank-aligned halves), drained by a single merged op per plane, all
    drains on ACT (DVE is the binding engine).
  Weights are generated on-chip via gpsimd memset + affine_select; no
  weight input tensor. ACT Sign with nonzero bias needs manually
  registered const APs (only 0.0/1.0 are pre-registered).
  Requires no x == th exactly (holds for this input distribution).
"""

import sys

import numpy as np

try:
    from concourse import bacc, bass, mybir, tile
    from concourse.bass_utils import run_bass_kernel_spmd
except ImportError:  # fresh grading dir: concourse lives in the trn repo
    sys.path.insert(0, "/opt/trn_rl_repo")
    from concourse import bacc, bass, mybir, tile
    from concourse.bass_utils import run_bass_kernel_spmd

B, N = 2048, 32768
NCORES = 8
ROWS = B // NCORES          # 256 rows per core
NB = N // 8                 # 4096 output bytes per row per threshold
P = 128                     # partitions
FT = 8192                   # free-dim tile of x (f32) per inner iteration
GT = FT // 8                # bytes per row per tile-plane = 1024
CHUNK = 512                 # matmul free dim (one PSUM bank)

DRAIN = ("act", "act", "act")   # PSUM drain engine per plane
XBUFS, BBUFS, OBUFS, PSBUFS = 2, 6, 2, 4  # psum tiles are 2 banks each

_cache: dict = {}


def _build(
    ths,
    loop: int = 1,
    drain_eng=DRAIN,
    xbufs: int = XBUFS,
    bbufs: int = BBUFS,
    obufs: int = OBUFS,
    psbufs: int = PSBUFS,
) -> "bass.Bass":
    nc = bacc.Bacc()
    # const APs for ACT Sign biases (only 0.0/1.0 pre-registered by bacc)
    for th in sorted({-float(t) for t in ths} - {0.0, 1.0}):
        cts = nc.alloc_sbuf_tensor(f"const-f32-{th}", [P, 1], mybir.dt.float32)
        nc.gpsimd.memset(cts.ap(), th)
        nc.const_aps.aps[(mybir.dt.float32, th)] = cts.ap()
    nc.all_engine_barrier()

    x_in = nc.declare_dram_parameter("x", [ROWS, N], mybir.dt.float32, isOutput=False)
    out_ext = nc.declare_dram_parameter(
        "out", [ROWS, 3, NB], mybir.dt.uint8, isOutput=True
    )
    out_flat = out_ext.ap().rearrange("r d g -> r (d g)")

    def make_w(wtile):
        # 16 half-blocks of 128 cols; DoubleRow pairs consecutive halves.
        # blocks 0-7: 2^(7-i) * I ({0,1} bits); 8-15: 2^(6-i) * I (Sign).
        for b in range(16):
            s = float(2 ** (7 - b)) if b < 8 else float(2.0 ** (6 - (b - 8)))
            blk = wtile[:, b * P : (b + 1) * P]
            nc.gpsimd.memset(blk, s)
            nc.gpsimd.affine_select(
                out=blk, in_=blk, pattern=[[1, P]],
                compare_op=mybir.AluOpType.is_equal, fill=0.0,
                base=0, channel_multiplier=-1,
            )

    def body(tc, wtile, xpool, bpool, opool, pspool):
        wv = wtile.rearrange("p (j h m) -> p j h m", j=8, h=2)  # DR pair view
        for pb in range(ROWS // P):
            r0 = pb * P
            ob = opool.tile([P, 3 * NB], mybir.dt.uint8, name="ob", tag="ob")
            for fti in range(N // FT):
                c0 = fti * FT
                g0 = c0 // 8
                nchunks = GT // CHUNK
                xt = xpool.tile([P, FT], mybir.dt.float32, name="xt", tag="xt")
                nc.sync.dma_start(out=xt[:], in_=x_in[r0 : r0 + P, c0 : c0 + FT])

                bvs = []
                for t in range(3):
                    bits = bpool.tile(
                        [P, FT], mybir.dt.float8e4, name="bits", tag="bits"
                    )
                    if t == 1:
                        nc.scalar.activation(
                            out=bits[:], in_=xt[:],
                            func=mybir.ActivationFunctionType.Sign,
                            bias=-ths[t],
                        )
                    else:
                        nc.vector.tensor_scalar(
                            out=bits[:], in0=xt[:], scalar1=ths[t],
                            scalar2=None, op0=mybir.AluOpType.is_gt,
                        )
                    # bit-pair view: [p, chunk, pairidx j, pair elem, byte]
                    bvs.append(
                        bits.rearrange("p (c g f e) -> p c f e g", g=CHUNK, f=4, e=2)
                    )

                # one 2-bank PSUM tile per plane; single merged drain
                pss = {
                    t: pspool.tile([P, GT], mybir.dt.float32, name="ps", tag="ps")
                    for t in range(3)
                }
                for j in range(4):
                    for t in (0, 2, 1):  # planes 0/2 share weight blocks:
                        # adjacency halves LDWEIGHTS switches (8/tile not 12)
                        enc = 4 if t == 1 else 0  # Sign weight blocks at 4+j
                        for c in range(nchunks):
                            nc.tensor.matmul(
                                pss[t][:, c * CHUNK : (c + 1) * CHUNK],
                                wv[:, enc + j, :, :],
                                bvs[t][:, c, j, :, :],
                                start=(j == 0), stop=(j == 3),
                                perf_mode=mybir.MatmulPerfMode.DoubleRow,
                            )
                for t, ps in pss.items():
                    dst = ob[:, t * NB + g0 : t * NB + g0 + GT]
                    eng = drain_eng[t]
                    C = 127.5 if t == 1 else 0.0  # Sign-plane offset
                    if C == 0.0:
                        if eng == "dve":
                            nc.vector.tensor_copy(out=dst, in_=ps[:])
                        else:
                            nc.scalar.copy(out=dst, in_=ps[:])
                    elif eng == "dve":
                        nc.vector.tensor_scalar(
                            out=dst, in0=ps[:], scalar1=C, scalar2=None,
                            op0=mybir.AluOpType.add,
                        )
                    else:
                        nc.scalar.activation(
                            out=dst, in_=ps[:],
                            func=mybir.ActivationFunctionType.Copy,
                            bias=C,
                        )
            nc.sync.dma_start(out=out_flat[r0 : r0 + P, :], in_=ob[:])

    with tile.TileContext(nc) as tc:
        with (
            tc.tile_pool(name="wpool", bufs=1) as wpool,
            tc.tile_pool(name="xpool", bufs=xbufs) as xpool,
            tc.tile_pool(name="bpool", bufs=bbufs) as bpool,
            tc.tile_pool(name="opool", bufs=obufs) as opool,
            tc.tile_pool(name="psum", bufs=psbufs, space="PSUM") as pspool,
        ):
            wtile = wpool.tile([P, 16 * P], mybir.dt.float8e4)
            make_w(wtile)
            if loop == 1:
                body(tc, wtile, xpool, bpool, opool, pspool)
            else:
                with tc.For_i(0, loop, 1):
                    body(tc, wtile, xpool, bpool, opool, pspool)
    nc.compile()
    return nc


def kernel(x: np.ndarray, depth_ths: np.ndarray) -> np.ndarray:
    x = np.asarray(x)
    ths = tuple(float(v) for v in np.asarray(depth_ths, dtype=np.float32))
    assert x.shape == (B, N) and len(ths) == 3

    if ths not in _cache:
        _cache[ths] = _build(ths)
    nc = _cache[ths]

    in_maps = [
        {"x": np.ascontiguousarray(x[i * ROWS : (i + 1) * ROWS])}
        for i in range(NCORES)
    ]
    res = run_bass_kernel_spmd(nc, in_maps, list(range(NCORES)))
    return np.concatenate([res.results[i]["out"] for i in range(NCORES)], axis=0)


# revision 10
# speedup vs baseline: 1.1921x; 1.0230x over previous
"""Binarize kernel for Trainium2: out[b, d, n/8] = packbits(x[b, :] > th[d]).

x: [2048, 32768] f32. depth_ths: [3] f32. out: [2048, 3, 4096] uint8.
8-way data parallel over batch (256 rows/core).

Architecture (v4 — DoubleRow bitpack; measured best of 4 generations):
  The kernel is NOT DMA-bound (DMA floor ~60-90 us/core): the binding
  resources are the elementwise engines and the PE moving-operand fetch.
  - Compares stay CONTIGUOUS (fastest form on both engines): planes
    0/2 (th=-0.67/+0.67) on DVE tensor_scalar is_gt (~274 G elem/s,
    2x_2P mode), plane 1 (th=0.0) on ACT Sign (~145 G; {-1,+1}
    encoding, halved matmul weights + 127.5 folded into the drain).
  - Bit-packing: byte[g] = sum_i 2^(7-i) bits[8g+i] via PE matmuls with
    scaled-identity fp8 stationary weights. perf_mode=DoubleRow packs
    the two ADJACENT bits (2j, 2j+1) of each byte per PE cell: 4
    DoubleRow matmuls per 512-byte chunk instead of 8 stride-8 matmuls
    (which fetch at only ~1 col/cycle). Stationary = [w_2j || w_2j+1]
    viewed [p, 2, 128]; moving = bit-pairs viewed [p, 2, 512].
    Measured -14% end-to-end vs the deinterleaved-sub-compare v3.
  - PSUM: one 2-bank [128, 1024] f32 tile per plane (matmuls write
    bank-aligned halves), drained by a single merged op per plane, all
    drains on ACT (DVE is the binding engine).
  Weights are generated on-chip via gpsimd memset + affine_select; no
  weight input tensor. ACT Sign with nonzero bias needs manually
  registered const APs (only 0.0/1.0 are pre-registered).
  Requires no x == th exactly (holds for this input distribution).
"""

import sys

import numpy as np

try:
    from concourse import bacc, bass, mybir, tile
    from concourse.bass_utils import run_bass_kernel_spmd
except ImportError:  # fresh grading dir: concourse lives in the trn repo
    sys.path.insert(0, "/opt/trn_rl_repo")
    from concourse import bacc, bass, mybir, tile
    from concourse.bass_utils import run_bass_kernel_spmd

B, N = 2048, 32768
NCORES = 8
ROWS = B // NCORES          # 256 rows per core
NB = N // 8                 # 4096 output bytes per row per threshold
P = 128                     # partitions
FT = 8192                   # free-dim tile of x (f32) per inner iteration
GT = FT // 8                # bytes per row per tile-plane = 1024
CHUNK = 512                 # matmul free dim (one PSUM bank)

DRAIN = ("act", "act", "act")   # PSUM drain engine per plane
DVE_TILES1 = 4                  # plane-1 tiles (of 8) compared on DVE
XBUFS, BBUFS, OBUFS, PSBUFS = 2, 6, 2, 4  # psum tiles are 2 banks each

_cache: dict = {}


def _build(
    ths,
    loop: int = 1,
    drain_eng=DRAIN,
    xbufs: int = XBUFS,
    bbufs: int = BBUFS,
    obufs: int = OBUFS,
    psbufs: int = PSBUFS,
) -> "bass.Bass":
    nc = bacc.Bacc()
    # const APs for ACT Sign biases (only 0.0/1.0 pre-registered by bacc)
    for th in sorted({-float(t) for t in ths} - {0.0, 1.0}):
        cts = nc.alloc_sbuf_tensor(f"const-f32-{th}", [P, 1], mybir.dt.float32)
        nc.gpsimd.memset(cts.ap(), th)
        nc.const_aps.aps[(mybir.dt.float32, th)] = cts.ap()
    nc.all_engine_barrier()

    x_in = nc.declare_dram_parameter("x", [ROWS, N], mybir.dt.float32, isOutput=False)
    out_ext = nc.declare_dram_parameter(
        "out", [ROWS, 3, NB], mybir.dt.uint8, isOutput=True
    )
    out_flat = out_ext.ap().rearrange("r d g -> r (d g)")

    def make_w(wtile):
        # 16 half-blocks of 128 cols; DoubleRow pairs consecutive halves.
        # blocks 0-7: 2^(7-i) * I ({0,1} bits); 8-15: 2^(6-i) * I (Sign).
        for b in range(16):
            s = float(2 ** (7 - b)) if b < 8 else float(2.0 ** (6 - (b - 8)))
            blk = wtile[:, b * P : (b + 1) * P]
            nc.gpsimd.memset(blk, s)
            nc.gpsimd.affine_select(
                out=blk, in_=blk, pattern=[[1, P]],
                compare_op=mybir.AluOpType.is_equal, fill=0.0,
                base=0, channel_multiplier=-1,
            )

    def body(tc, wtile, xpool, bpool, opool, pspool):
        wv = wtile.rearrange("p (j h m) -> p j h m", j=8, h=2)  # DR pair view
        for pb in range(ROWS // P):
            r0 = pb * P
            ob = opool.tile([P, 3 * NB], mybir.dt.uint8, name="ob", tag="ob")
            for fti in range(N // FT):
                c0 = fti * FT
                g0 = c0 // 8
                nchunks = GT // CHUNK
                xt = xpool.tile([P, FT], mybir.dt.float32, name="xt", tag="xt")
                nc.sync.dma_start(out=xt[:], in_=x_in[r0 : r0 + P, c0 : c0 + FT])

                # plane 1 engine-split at tile granularity: DVE_TILES1 of 8
                # tiles compare on DVE ({0,1} weights), rest on ACT Sign —
                # balances ACT (Sign + drains) against DVE's slack.
                tg = pb * (N // FT) + fti
                p1_dve = (tg * DVE_TILES1) % 8 < DVE_TILES1
                bvs = []
                for t in range(3):
                    bits = bpool.tile(
                        [P, FT], mybir.dt.float8e4, name="bits", tag="bits"
                    )
                    if t == 1 and not p1_dve:
                        nc.scalar.activation(
                            out=bits[:], in_=xt[:],
                            func=mybir.ActivationFunctionType.Sign,
                            bias=-ths[t],
                        )
                    else:
                        nc.vector.tensor_scalar(
                            out=bits[:], in0=xt[:], scalar1=ths[t],
                            scalar2=None, op0=mybir.AluOpType.is_gt,
                        )
                    # bit-pair view: [p, chunk, pairidx j, pair elem, byte]
                    bvs.append(
                        bits.rearrange("p (c g f e) -> p c f e g", g=CHUNK, f=4, e=2)
                    )

                # one 2-bank PSUM tile per plane; single merged drain
                pss = {
                    t: pspool.tile([P, GT], mybir.dt.float32, name="ps", tag="ps")
                    for t in range(3)
                }
                for j in range(4):
                    for t in (0, 2, 1):  # planes 0/2 share weight blocks:
                        # adjacency halves LDWEIGHTS switches (8/tile not 12)
                        enc = 4 if (t == 1 and not p1_dve) else 0
                        for c in range(nchunks):
                            nc.tensor.matmul(
                                pss[t][:, c * CHUNK : (c + 1) * CHUNK],
                                wv[:, enc + j, :, :],
                                bvs[t][:, c, j, :, :],
                                start=(j == 0), stop=(j == 3),
                                perf_mode=mybir.MatmulPerfMode.DoubleRow,
                            )
                for t, ps in pss.items():
                    dst = ob[:, t * NB + g0 : t * NB + g0 + GT]
                    eng = drain_eng[t]
                    C = 127.5 if (t == 1 and not p1_dve) else 0.0
                    if C == 0.0:
                        if eng == "dve":
                            nc.vector.tensor_copy(out=dst, in_=ps[:])
                        else:
                            nc.scalar.copy(out=dst, in_=ps[:])
                    elif eng == "dve":
                        nc.vector.tensor_scalar(
                            out=dst, in0=ps[:], scalar1=C, scalar2=None,
                            op0=mybir.AluOpType.add,
                        )
                    else:
                        nc.scalar.activation(
                            out=dst, in_=ps[:],
                            func=mybir.ActivationFunctionType.Copy,
                            bias=C,
                        )
            nc.sync.dma_start(out=out_flat[r0 : r0 + P, :], in_=ob[:])

    with tile.TileContext(nc) as tc:
        with (
            tc.tile_pool(name="wpool", bufs=1) as wpool,
            tc.tile_pool(name="xpool", bufs=xbufs) as xpool,
            tc.tile_pool(name="bpool", bufs=bbufs) as bpool,
            tc.tile_pool(name="opool", bufs=obufs) as opool,
            tc.tile_pool(name="psum", bufs=psbufs, space="PSUM") as pspool,
        ):
            wtile = wpool.tile([P, 16 * P], mybir.dt.float8e4)
            make_w(wtile)
            if loop == 1:
                body(tc, wtile, xpool, bpool, opool, pspool)
            else:
                with tc.For_i(0, loop, 1):
                    body(tc, wtile, xpool, bpool, opool, pspool)
    nc.compile()
    return nc


def kernel(x: np.ndarray, depth_ths: np.ndarray) -> np.ndarray:
    x = np.asarray(x)
    ths = tuple(float(v) for v in np.asarray(depth_ths, dtype=np.float32))
    assert x.shape == (B, N) and len(ths) == 3

    if ths not in _cache:
        _cache[ths] = _build(ths)
    nc = _cache[ths]

    in_maps = [
        {"x": np.ascontiguousarray(x[i * ROWS : (i + 1) * ROWS])}
        for i in range(NCORES)
    ]
    res = run_bass_kernel_spmd(nc, in_maps, list(range(NCORES)))
    return np.concatenate([res.results[i]["out"] for i in range(NCORES)], axis=0)


# revision 12
# speedup vs baseline: 1.2603x; 1.0572x over previous
"""Binarize kernel for Trainium2: out[b, d, n/8] = packbits(x[b, :] > th[d]).

x: [2048, 32768] f32. depth_ths: [3] f32. out: [2048, 3, 4096] uint8.
8-way data parallel over batch (256 rows/core).

Architecture (v4 — DoubleRow bitpack; measured best of 4 generations):
  The kernel is NOT DMA-bound (DMA floor ~60-90 us/core): the binding
  resources are the elementwise engines and the PE moving-operand fetch.
  - Compares stay CONTIGUOUS (fastest form on both engines): planes
    0/2 (th=-0.67/+0.67) on DVE tensor_scalar is_gt (~274 G elem/s,
    2x_2P mode), plane 1 (th=0.0) on ACT Sign (~145 G; {-1,+1}
    encoding, halved matmul weights + 127.5 folded into the drain).
  - Bit-packing: byte[g] = sum_i 2^(7-i) bits[8g+i] via PE matmuls with
    scaled-identity fp8 stationary weights. perf_mode=DoubleRow packs
    the two ADJACENT bits (2j, 2j+1) of each byte per PE cell: 4
    DoubleRow matmuls per 512-byte chunk instead of 8 stride-8 matmuls
    (which fetch at only ~1 col/cycle). Stationary = [w_2j || w_2j+1]
    viewed [p, 2, 128]; moving = bit-pairs viewed [p, 2, 512].
    Measured -14% end-to-end vs the deinterleaved-sub-compare v3.
  - PSUM: one 2-bank [128, 1024] f32 tile per plane (matmuls write
    bank-aligned halves), drained by a single merged op per plane, all
    drains on ACT (DVE is the binding engine).
  Weights (scaled identities, fp8, 256KB) are DMA'd from a host-built
  tensor: ~0.7 us on the DMA timeline vs ~60+ us of serial gpsimd
  generation on the one-shot execution path. ACT Sign with nonzero bias
  needs manually registered const APs (only 0.0/1.0 are pre-registered).
  Requires no x == th exactly (holds for this input distribution).
"""

import sys

import numpy as np

try:
    from concourse import bacc, bass, mybir, tile
    from concourse.bass_utils import run_bass_kernel_spmd
except ImportError:  # fresh grading dir: concourse lives in the trn repo
    sys.path.insert(0, "/opt/trn_rl_repo")
    from concourse import bacc, bass, mybir, tile
    from concourse.bass_utils import run_bass_kernel_spmd

B, N = 2048, 32768
NCORES = 8
ROWS = B // NCORES          # 256 rows per core
NB = N // 8                 # 4096 output bytes per row per threshold
P = 128                     # partitions
FT = 8192                   # free-dim tile of x (f32) per inner iteration
GT = FT // 8                # bytes per row per tile-plane = 1024
CHUNK = 512                 # matmul free dim (one PSUM bank)

DRAIN = ("act", "act", "act")   # PSUM drain engine per plane
DVE_TILES1 = 4                  # plane-1 tiles (of 8) compared on DVE
XBUFS, BBUFS, OBUFS, PSBUFS = 2, 6, 2, 4  # psum tiles are 2 banks each

_cache: dict = {}


def _build(
    ths,
    loop: int = 1,
    drain_eng=DRAIN,
    xbufs: int = XBUFS,
    bbufs: int = BBUFS,
    obufs: int = OBUFS,
    psbufs: int = PSBUFS,
) -> "bass.Bass":
    nc = bacc.Bacc()
    # const APs for ACT Sign biases (only 0.0/1.0 pre-registered by bacc)
    for th in sorted({-float(t) for t in ths} - {0.0, 1.0}):
        cts = nc.alloc_sbuf_tensor(f"const-f32-{th}", [P, 1], mybir.dt.float32)
        nc.gpsimd.memset(cts.ap(), th)
        nc.const_aps.aps[(mybir.dt.float32, th)] = cts.ap()
    nc.all_engine_barrier()

    x_in = nc.declare_dram_parameter("x", [ROWS, N], mybir.dt.float32, isOutput=False)
    w_in = nc.declare_dram_parameter(
        "w", [P, 16 * P], mybir.dt.float8e4, isOutput=False
    )
    out_ext = nc.declare_dram_parameter(
        "out", [ROWS, 3, NB], mybir.dt.uint8, isOutput=True
    )
    out_flat = out_ext.ap().rearrange("r d g -> r (d g)")

    def body(tc, wtile, xpool, bpool, opool, pspool):
        wv = wtile.rearrange("p (j h m) -> p j h m", j=8, h=2)  # DR pair view
        for pb in range(ROWS // P):
            r0 = pb * P
            ob = opool.tile([P, 3 * NB], mybir.dt.uint8, name="ob", tag="ob")
            for fti in range(N // FT):
                c0 = fti * FT
                g0 = c0 // 8
                nchunks = GT // CHUNK
                xt = xpool.tile([P, FT], mybir.dt.float32, name="xt", tag="xt")
                nc.sync.dma_start(out=xt[:], in_=x_in[r0 : r0 + P, c0 : c0 + FT])

                # plane 1 engine-split at tile granularity: DVE_TILES1 of 8
                # tiles compare on DVE ({0,1} weights), rest on ACT Sign —
                # balances ACT (Sign + drains) against DVE's slack.
                tg = pb * (N // FT) + fti
                p1_dve = (tg * DVE_TILES1) % 8 < DVE_TILES1
                bvs = []
                for t in range(3):
                    bits = bpool.tile(
                        [P, FT], mybir.dt.float8e4, name="bits", tag="bits"
                    )
                    if t == 1 and not p1_dve:
                        nc.scalar.activation(
                            out=bits[:], in_=xt[:],
                            func=mybir.ActivationFunctionType.Sign,
                            bias=-ths[t],
                        )
                    else:
                        nc.vector.tensor_scalar(
                            out=bits[:], in0=xt[:], scalar1=ths[t],
                            scalar2=None, op0=mybir.AluOpType.is_gt,
                        )
                    # bit-pair view: [p, chunk, pairidx j, pair elem, byte]
                    bvs.append(
                        bits.rearrange("p (c g f e) -> p c f e g", g=CHUNK, f=4, e=2)
                    )

                # one 2-bank PSUM tile per plane; single merged drain
                pss = {
                    t: pspool.tile([P, GT], mybir.dt.float32, name="ps", tag="ps")
                    for t in range(3)
                }
                for j in range(4):
                    for t in (0, 2, 1):  # planes 0/2 share weight blocks:
                        # adjacency halves LDWEIGHTS switches (8/tile not 12)
                        enc = 4 if (t == 1 and not p1_dve) else 0
                        for c in range(nchunks):
                            nc.tensor.matmul(
                                pss[t][:, c * CHUNK : (c + 1) * CHUNK],
                                wv[:, enc + j, :, :],
                                bvs[t][:, c, j, :, :],
                                start=(j == 0), stop=(j == 3),
                                perf_mode=mybir.MatmulPerfMode.DoubleRow,
                            )
                for t, ps in pss.items():
                    dst = ob[:, t * NB + g0 : t * NB + g0 + GT]
                    eng = drain_eng[t]
                    C = 127.5 if (t == 1 and not p1_dve) else 0.0
                    if C == 0.0:
                        if eng == "dve":
                            nc.vector.tensor_copy(out=dst, in_=ps[:])
                        else:
                            nc.scalar.copy(out=dst, in_=ps[:])
                    elif eng == "dve":
                        nc.vector.tensor_scalar(
                            out=dst, in0=ps[:], scalar1=C, scalar2=None,
                            op0=mybir.AluOpType.add,
                        )
                    else:
                        nc.scalar.activation(
                            out=dst, in_=ps[:],
                            func=mybir.ActivationFunctionType.Copy,
                            bias=C,
                        )
            nc.sync.dma_start(out=out_flat[r0 : r0 + P, :], in_=ob[:])

    with tile.TileContext(nc) as tc:
        with (
            tc.tile_pool(name="wpool", bufs=1) as wpool,
            tc.tile_pool(name="xpool", bufs=xbufs) as xpool,
            tc.tile_pool(name="bpool", bufs=bbufs) as bpool,
            tc.tile_pool(name="opool", bufs=obufs) as opool,
            tc.tile_pool(name="psum", bufs=psbufs, space="PSUM") as pspool,
        ):
            # DMA'd weights: ~0.7 us vs ~60+ us of serial gpsimd
            # generation on the one-shot path (gpsimd is ~9 G elem/s).
            wtile = wpool.tile([P, 16 * P], mybir.dt.float8e4)
            nc.sync.dma_start(out=wtile[:], in_=w_in[:])
            if loop == 1:
                body(tc, wtile, xpool, bpool, opool, pspool)
            else:
                with tc.For_i(0, loop, 1):
                    body(tc, wtile, xpool, bpool, opool, pspool)
    nc.compile()
    return nc


def _weights() -> np.ndarray:
    # 16 half-blocks of 128 cols; DoubleRow pairs consecutive halves.
    # blocks 0-7: 2^(7-i) * I ({0,1} bits); 8-15: 2^(6-i) * I (Sign).
    import ml_dtypes

    dt = ml_dtypes.float8_e4m3fn
    w = np.zeros((P, 16 * P), dtype=dt)
    for b in range(16):
        s = float(2 ** (7 - b)) if b < 8 else float(2.0 ** (6 - (b - 8)))
        np.fill_diagonal(w[:, b * P : (b + 1) * P], dt(s))
    return w


def kernel(x: np.ndarray, depth_ths: np.ndarray) -> np.ndarray:
    x = np.asarray(x)
    ths = tuple(float(v) for v in np.asarray(depth_ths, dtype=np.float32))
    assert x.shape == (B, N) and len(ths) == 3

    if ths not in _cache:
        _cache[ths] = _build(ths)
    nc = _cache[ths]

    w = _weights()
    in_maps = [
        {"x": np.ascontiguousarray(x[i * ROWS : (i + 1) * ROWS]), "w": w}
        for i in range(NCORES)
    ]
    res = run_bass_kernel_spmd(nc, in_maps, list(range(NCORES)))
    return np.concatenate([res.results[i]["out"] for i in range(NCORES)], axis=0)
